# revision 8
# baseline (speedup 1.0000x reference)
"""Causal depthwise-conv self-attention kernel for Trainium2 (8 NeuronCores).

Math: out[b,t,d] = sum_i sum_k X[b,t-i,k] * W[i*D+d,k]   (i in 0..kW-1, zero for t<i)

Sharding: 8 cores = 2 batches x 4 channel-groups (256 output channels each).

Algorithm: 2-parallel fast-FIR (FFA) decomposition of the 4-tap conv.
Split time into even/odd phases x_e[k]=x[2k], x_o[k]=x[2k+1] and the taps
into 2-tap subfilters H0={W0,W2} (even taps), H1={W1,W3} (odd taps):
    F0 = H0 * x_e,   F1 = H1 * x_o,   F2 = (H0+H1) * (x_e+x_o)
    y_even[k] = F0[k] + F1[k-1]
    y_odd[k]  = F2[k] - F0[k] - F1[k]
This computes the conv with 3 half-rate 2-tap subfilters = 3/4 of the
direct matmul work (384 instead of 512 PE matmuls per core); the
recombines are cheap vector-engine adds fully hidden under the PE stream.
bf16 operands (FWL weight loads, half DMA), fp32 PSUM accumulation.
Host does layout only: phase de-interleave / transpose / bf16 cast on the
way in, phase re-interleave + transpose on the way out.
"""

import numpy as np

import concourse.bacc as bacc
import concourse.mybir as mybir
import concourse.tile as tile
from concourse.bass_utils import run_bass_kernel_spmd

# bass_utils imports antenv.axon_hooks when BASS_TRACE is set; that module is
# absent from this image. Provide a no-op stand-in so tracing degrades
# gracefully instead of crashing the run.
try:
    import antenv.axon_hooks  # noqa: F401
except ImportError:
    import sys
    import types

    import antenv

    _hooks = types.ModuleType("antenv.axon_hooks")
    _hooks._h = None
    _hooks.set_axon_ntff_profile_hook = lambda h: setattr(_hooks, "_h", h)
    _hooks.get_axon_ntff_profile_hook = lambda: _hooks._h
    sys.modules["antenv.axon_hooks"] = _hooks
    antenv.axon_hooks = _hooks

BSZ, T, D, KW = 2, 4096, 1024, 4
NCORES = 8
CGROUPS = 4            # channel groups (one per core within a batch)
CPG = D // CGROUPS     # channels per core = 256
KC = D // 128          # contraction chunks = 8
HT = T // 2            # half-rate stream length = 2048
KT = HT // 512         # k-tiles of 512 per phase = 4
CS = CPG // 128        # channel subtiles per core = 2
WARMUP_MMS = 12        # PE busy-burst during initial DMA (flips HAM to 8/8)

_last_results = None   # test harness peeks at this for profiling info
_nc_cache = None       # compiled program reused across kernel() calls

F32 = mybir.dt.float32
BF16 = mybir.dt.bfloat16
MULT = mybir.AluOpType.mult
ADD = mybir.AluOpType.add
BYPASS = mybir.AluOpType.bypass


def _build_nc():
    nc = bacc.Bacc(trn_type="TRN2", enable_partition_id=False)
    # half-rate phase streams, 1 zero halo column at the left
    xe = nc.dram_tensor("xe", [128, KC, 1 + HT], BF16, kind="ExternalInput")
    xo = nc.dram_tensor("xo", [128, KC, 1 + HT], BF16, kind="ExternalInput")
    wt = nc.dram_tensor("wt", [128, KC, KW, CPG], BF16, kind="ExternalInput")
    # phase-major output; host re-interleaves time
    out_ct = nc.dram_tensor("out_ct", [CS, 128, 2, HT], F32,
                            kind="ExternalOutput")

    groups = [(kt, cs) for kt in range(KT) for cs in range(CS)]

    with tile.TileContext(nc) as tc:
        with (
            tc.tile_pool(name="xpool", bufs=1) as xpool,
            tc.tile_pool(name="wpool", bufs=1) as wpool,
            tc.tile_pool(name="fpool", bufs=4) as fpool,
            tc.tile_pool(name="opool", bufs=6) as opool,
            tc.tile_pool(name="psum", bufs=6, space="PSUM") as psum_pool,
        ):
            xe_sb = xpool.tile([128, KC, 1 + HT], BF16, name="xe_sb")
            xo_sb = xpool.tile([128, KC, 1 + HT], BF16, name="xo_sb")
            s_sb = xpool.tile([128, KC, 1 + HT], BF16, name="s_sb")
            wt_sb = wpool.tile([128, KC, KW, CPG], BF16, name="wt_sb")
            ws_sb = wpool.tile([128, KC, 2, CPG], BF16, name="ws_sb")
            zcol = wpool.tile([128, 1], F32, name="zcol")
            dummy = wpool.tile([128, 512], BF16, name="dummy")

            # vector queue head: memset the warmup tile so the PE can start
            # its HAM warm-up burst as soon as the framework preamble ends.
            nc.vector.memset(dummy[:].bitcast(mybir.dt.uint16), 0)
            nc.gpsimd.memset(zcol[:], 0.0)

            # --- input DMA (issue in first-needed order) ---
            # weights on the scalar ring, kc-ordered (stays ahead of the PE)
            for kc in range(KC):
                nc.scalar.dma_start(wt_sb[:, kc], wt[:, kc])
            # phase streams: xe on sync ring, xo on gpsimd ring, kt-window
            # ordered so the first matmul group's inputs land first.
            # Windows are EXCLUSIVE ([0,513), [513,1025), ...) so no matmul
            # read ever intersects a later window's DMA (no forward deps),
            # and merged per-kt (dispatch is ~600ns per DMA instruction) with
            # kt0 split per-kc so the first matmuls can start early.
            for kc in range(KC):
                nc.sync.dma_start(xe_sb[:, kc, 0:513], xe[:, kc, 0:513])
                nc.gpsimd.dma_start(xo_sb[:, kc, 0:513], xo[:, kc, 0:513])
            for kt in range(1, KT):
                lo, hi = kt * 512 + 1, (kt + 1) * 512 + 1
                nc.sync.dma_start(xe_sb[:, :, lo:hi], xe[:, :, lo:hi])
                nc.gpsimd.dma_start(xo_sb[:, :, lo:hi], xo[:, :, lo:hi])

            # HAM warmup: keep PE busy while the first DMAs land.
            ps_w = psum_pool.tile([128, 512], F32, name="ps_warm", tag="ps")
            for w in range(WARMUP_MMS):
                nc.tensor.matmul(ps_w[:], dummy[:, :128], dummy[:],
                                 start=True, stop=True, skip_group_check=True)

            # s = x_e + x_o, computed per (kt-window, kc) on the vector engine
            def s_windows(kt, kcs=range(KC)):
                lo = kt * 512
                hi = min(1 + HT, lo + 513)
                for kc in kcs:
                    nc.vector.scalar_tensor_tensor(
                        out=s_sb[:, kc, lo:hi], in0=xe_sb[:, kc, lo:hi],
                        scalar=0.0, in1=xo_sb[:, kc, lo:hi],
                        op0=BYPASS, op1=ADD)

            # subfilter weight sums (vector; gpsimd lacks TensorScalarPtr):
            # ws[:,kc,0] = W0+W1, ws[:,kc,1] = W2+W3 — interleaved per kc
            # with the first s-window batch to match DMA arrival order.
            for kc in range(KC):
                nc.vector.scalar_tensor_tensor(
                    out=ws_sb[:, kc, 0], in0=wt_sb[:, kc, 0], scalar=0.0,
                    in1=wt_sb[:, kc, 1], op0=BYPASS, op1=ADD)
                nc.vector.scalar_tensor_tensor(
                    out=ws_sb[:, kc, 1], in0=wt_sb[:, kc, 2], scalar=0.0,
                    in1=wt_sb[:, kc, 3], op0=BYPASS, op1=ADD)
                s_windows(0, [kc])
            s_windows(1)

            f1_prev = {cs: None for cs in range(CS)}  # per-cs previous F1 sbuf

            for g, (kt, cs) in enumerate(groups):
                k0 = kt * 512
                ps = {j: psum_pool.tile([128, 512], F32,
                                        name=f"F{j}_g{g}", tag="ps")
                      for j in range(3)}
                wcol = slice(cs * 128, (cs + 1) * 128)
                for kc in range(KC):
                    # F0 += W0 @ x_e[k] ; F0 += W2 @ x_e[k-1]
                    nc.tensor.matmul(ps[0][:], wt_sb[:, kc, 0, wcol],
                                     xe_sb[:, kc, 1 + k0:1 + k0 + 512],
                                     start=(kc == 0), stop=False)
                    nc.tensor.matmul(ps[0][:], wt_sb[:, kc, 2, wcol],
                                     xe_sb[:, kc, k0:k0 + 512],
                                     start=False, stop=(kc == KC - 1))
                    # F1 += W1 @ x_o[k] ; F1 += W3 @ x_o[k-1]
                    nc.tensor.matmul(ps[1][:], wt_sb[:, kc, 1, wcol],
                                     xo_sb[:, kc, 1 + k0:1 + k0 + 512],
                                     start=(kc == 0), stop=False)
                    nc.tensor.matmul(ps[1][:], wt_sb[:, kc, 3, wcol],
                                     xo_sb[:, kc, k0:k0 + 512],
                                     start=False, stop=(kc == KC - 1))
                    # F2 += (W0+W1) @ s[k] ; F2 += (W2+W3) @ s[k-1]
                    nc.tensor.matmul(ps[2][:], ws_sb[:, kc, 0, wcol],
                                     s_sb[:, kc, 1 + k0:1 + k0 + 512],
                                     start=(kc == 0), stop=False)
                    nc.tensor.matmul(ps[2][:], ws_sb[:, kc, 1, wcol],
                                     s_sb[:, kc, k0:k0 + 512],
                                     start=False, stop=(kc == KC - 1))

                # keep the s pipeline ahead of the matmuls for the next kt
                if cs == 0 and kt + 2 < KT:
                    s_windows(kt + 2)

                # --- recombine ---
                # (DVE reads at most one PSUM operand per op, so F0/F1 are
                # staged to SBUF by the otherwise-idle scalar engine)
                # f1[:, 1+j] = F1[k0+j]; f1[:, 0] = F1[k0-1] (prev tile)
                f1 = fpool.tile([128, 513], F32, name=f"f1_g{g}", tag="f1")
                f0 = fpool.tile([128, 512], F32, name=f"f0_g{g}", tag="f0")
                nc.scalar.copy(f1[:, 1:513], ps[1][:])
                prev = f1_prev[cs]
                if prev is None:
                    nc.scalar.copy(f1[:, 0:1], zcol[:])
                else:
                    nc.scalar.copy(f1[:, 0:1], prev[:, 512:513])
                f1_prev[cs] = f1
                nc.scalar.copy(f0[:], ps[0][:])

                oe = opool.tile([128, 512], F32, name=f"oe_g{g}", tag="ob")
                t1 = opool.tile([128, 512], F32, name=f"t1_g{g}", tag="ob")
                oo = opool.tile([128, 512], F32, name=f"oo_g{g}", tag="ob")
                # y_even = F0 + F1[k-1]
                nc.vector.scalar_tensor_tensor(
                    out=oe[:], in0=f0[:], scalar=0.0, in1=f1[:, 0:512],
                    op0=BYPASS, op1=ADD)
                # y_odd = F2 - F0 - F1
                nc.vector.scalar_tensor_tensor(
                    out=t1[:], in0=f0[:], scalar=-1.0, in1=ps[2][:],
                    op0=MULT, op1=ADD)
                nc.vector.scalar_tensor_tensor(
                    out=oo[:], in0=f1[:, 1:513], scalar=-1.0, in1=t1[:],
                    op0=MULT, op1=ADD)
                nc.scalar.dma_start(out_ct[cs, :, 0, k0:k0 + 512], oe[:])
                nc.scalar.dma_start(out_ct[cs, :, 1, k0:k0 + 512], oo[:])

    nc.compile()
    return nc


def kernel(X: np.ndarray, W: np.ndarray) -> np.ndarray:
    global _last_results
    import ml_dtypes
    bf16 = ml_dtypes.bfloat16
    X = np.ascontiguousarray(X, dtype=np.float32).astype(bf16)
    W = np.ascontiguousarray(W, dtype=np.float32).astype(bf16)

    # phase-split X^T per batch with 1-column causal zero halo:
    # xe[p, kc, 1+k] = X[b, 2k, kc*128+p], xo[p, kc, 1+k] = X[b, 2k+1, ...]
    xes, xos = [], []
    for b in range(BSZ):
        v = X[b].reshape(HT, 2, KC, 128)  # [k, phase, kc, p]
        xe = np.zeros((128, KC, 1 + HT), dtype=bf16)
        xo = np.zeros((128, KC, 1 + HT), dtype=bf16)
        xe[:, :, 1:] = v[:, 0].transpose(2, 1, 0)
        xo[:, :, 1:] = v[:, 1].transpose(2, 1, 0)
        xes.append(xe)
        xos.append(xo)

    # W per core: wt[p, kc, i, c] = W[i*D + cg*CPG + c, kc*128 + p]
    W4 = W.reshape(KW, D, KC, 128)  # [i, d, kc, p]
    wts = []
    for cg in range(CGROUPS):
        wt = W4[:, cg * CPG:(cg + 1) * CPG, :, :].transpose(3, 2, 0, 1)
        wts.append(np.ascontiguousarray(wt))

    global _nc_cache
    if _nc_cache is None:
        _nc_cache = _build_nc()
    nc = _nc_cache
    in_maps = [{"xe": xes[c // CGROUPS], "xo": xos[c // CGROUPS],
                "wt": wts[c % CGROUPS]} for c in range(NCORES)]
    _last_results = run_bass_kernel_spmd(nc, in_maps, core_ids=list(range(NCORES)))

    out = np.empty((BSZ, T, D), dtype=np.float32)
    for c in range(NCORES):
        b, cg = c // CGROUPS, c % CGROUPS
        shard = _last_results.results[c]["out_ct"]  # [CS, 128, 2, HT]
        for cs in range(CS):
            cols = slice(cg * CPG + cs * 128, cg * CPG + (cs + 1) * 128)
            # [128, 2, HT] -> t-interleaved [T, 128]
            out[b, :, cols] = shard[cs].transpose(2, 1, 0).reshape(T, 128)
    return out


# revision 9
# speedup vs baseline: 1.0125x; 1.0125x over previous
"""Causal depthwise-conv self-attention kernel for Trainium2 (8 NeuronCores).

Math: out[b,t,d] = sum_i sum_k X[b,t-i,k] * W[i*D+d,k]   (i in 0..kW-1, zero for t<i)

Sharding: 8 cores = 2 batches x 4 channel-groups (256 output channels each).

Algorithm: 2-parallel fast-FIR (FFA) decomposition of the 4-tap conv.
Split time into even/odd phases x_e[k]=x[2k], x_o[k]=x[2k+1] and the taps
into 2-tap subfilters H0={W0,W2} (even taps), H1={W1,W3} (odd taps):
    F0 = H0 * x_e,   F1 = H1 * x_o,   F2 = (H0+H1) * (x_e+x_o)
    y_even[k] = F0[k] + F1[k-1]
    y_odd[k]  = F2[k] - F0[k] - F1[k]
This computes the conv with 3 half-rate 2-tap subfilters = 3/4 of the
direct matmul work (384 instead of 512 PE matmuls per core); the
recombines are cheap vector-engine adds fully hidden under the PE stream.
bf16 operands (FWL weight loads, half DMA), fp32 PSUM accumulation.
Host does layout only: phase de-interleave / transpose / bf16 cast on the
way in, phase re-interleave + transpose on the way out.
"""

import numpy as np

import concourse.bacc as bacc
import concourse.mybir as mybir
import concourse.tile as tile
from concourse.bass_utils import run_bass_kernel_spmd

# bass_utils imports antenv.axon_hooks when BASS_TRACE is set; that module is
# absent from this image. Provide a no-op stand-in so tracing degrades
# gracefully instead of crashing the run.
try:
    import antenv.axon_hooks  # noqa: F401
except ImportError:
    import sys
    import types

    import antenv

    _hooks = types.ModuleType("antenv.axon_hooks")
    _hooks._h = None
    _hooks.set_axon_ntff_profile_hook = lambda h: setattr(_hooks, "_h", h)
    _hooks.get_axon_ntff_profile_hook = lambda: _hooks._h
    sys.modules["antenv.axon_hooks"] = _hooks
    antenv.axon_hooks = _hooks

BSZ, T, D, KW = 2, 4096, 1024, 4
NCORES = 8
CGROUPS = 4            # channel groups (one per core within a batch)
CPG = D // CGROUPS     # channels per core = 256
KC = D // 128          # contraction chunks = 8
HT = T // 2            # half-rate stream length = 2048
KT = HT // 512         # k-tiles of 512 per phase = 4
CS = CPG // 128        # channel subtiles per core = 2
WARMUP_MMS = 12        # PE busy-burst during initial DMA (flips HAM to 8/8)

_last_results = None   # test harness peeks at this for profiling info
_nc_cache = None       # compiled program reused across kernel() calls

F32 = mybir.dt.float32
BF16 = mybir.dt.bfloat16
MULT = mybir.AluOpType.mult
ADD = mybir.AluOpType.add
BYPASS = mybir.AluOpType.bypass


def _build_nc():
    nc = bacc.Bacc(trn_type="TRN2", enable_partition_id=False)
    # half-rate phase streams, 1 zero halo column at the left
    xe = nc.dram_tensor("xe", [128, KC, 1 + HT], BF16, kind="ExternalInput")
    xo = nc.dram_tensor("xo", [128, KC, 1 + HT], BF16, kind="ExternalInput")
    wt = nc.dram_tensor("wt", [128, KC, KW, CPG], BF16, kind="ExternalInput")
    # phase-major output; host re-interleaves time
    out_ct = nc.dram_tensor("out_ct", [CS, 128, 2, HT], F32,
                            kind="ExternalOutput")

    groups = [(kt, cs) for kt in range(KT) for cs in range(CS)]

    with tile.TileContext(nc) as tc:
        with (
            tc.tile_pool(name="xpool", bufs=1) as xpool,
            tc.tile_pool(name="wpool", bufs=1) as wpool,
            tc.tile_pool(name="fpool", bufs=4) as fpool,
            tc.tile_pool(name="opool", bufs=6) as opool,
            tc.tile_pool(name="psum", bufs=6, space="PSUM") as psum_pool,
        ):
            xe_sb = xpool.tile([128, KC, 1 + HT], BF16, name="xe_sb")
            xo_sb = xpool.tile([128, KC, 1 + HT], BF16, name="xo_sb")
            s_sb = xpool.tile([128, KC, 1 + HT], BF16, name="s_sb")
            wt_sb = wpool.tile([128, KC, KW, CPG], BF16, name="wt_sb")
            ws_sb = wpool.tile([128, KC, 2, CPG], BF16, name="ws_sb")
            zcol = wpool.tile([128, 1], F32, name="zcol")
            dummy = wpool.tile([128, 512], BF16, name="dummy")

            # vector queue head: memset the warmup tile so the PE can start
            # its HAM warm-up burst as soon as the framework preamble ends.
            nc.vector.memset(dummy[:].bitcast(mybir.dt.uint16), 0)
            nc.gpsimd.memset(zcol[:], 0.0)

            # --- input DMA (issue in first-needed order) ---
            # weights on the scalar ring, kc-ordered (stays ahead of the PE)
            for kc in range(KC):
                nc.scalar.dma_start(wt_sb[:, kc], wt[:, kc])
            # phase streams: xe on sync ring, xo on gpsimd ring, kt-window
            # ordered so the first matmul group's inputs land first.
            # Windows are EXCLUSIVE ([0,513), [513,1025), ...) so no matmul
            # read ever intersects a later window's DMA (no forward deps),
            # and merged per-kt (dispatch is ~600ns per DMA instruction) with
            # kt0 split per-kc so the first matmuls can start early.
            # kt0 split across BOTH rings so it fully lands before the HAM
            # warmup ends — any PE gap in the first group drops HAM to 4/8
            # for ~25us (observed), so the first group must never stall.
            for kc in range(4):
                nc.sync.dma_start(xe_sb[:, kc, 0:513], xe[:, kc, 0:513])
                nc.gpsimd.dma_start(xo_sb[:, kc, 0:513], xo[:, kc, 0:513])
            for kc in range(4, KC):
                nc.gpsimd.dma_start(xe_sb[:, kc, 0:513], xe[:, kc, 0:513])
                nc.sync.dma_start(xo_sb[:, kc, 0:513], xo[:, kc, 0:513])
            for kt in range(1, KT):
                lo, hi = kt * 512 + 1, (kt + 1) * 512 + 1
                nc.sync.dma_start(xe_sb[:, :, lo:hi], xe[:, :, lo:hi])
                nc.gpsimd.dma_start(xo_sb[:, :, lo:hi], xo[:, :, lo:hi])

            # HAM warmup: keep PE busy while the first DMAs land.
            ps_w = psum_pool.tile([128, 512], F32, name="ps_warm", tag="ps")
            for w in range(WARMUP_MMS):
                nc.tensor.matmul(ps_w[:], dummy[:, :128], dummy[:],
                                 start=True, stop=True, skip_group_check=True)

            # s = x_e + x_o, computed per (kt-window, kc) on the vector engine
            def s_windows(kt, kcs=range(KC)):
                lo = kt * 512
                hi = min(1 + HT, lo + 513)
                for kc in kcs:
                    nc.vector.scalar_tensor_tensor(
                        out=s_sb[:, kc, lo:hi], in0=xe_sb[:, kc, lo:hi],
                        scalar=0.0, in1=xo_sb[:, kc, lo:hi],
                        op0=BYPASS, op1=ADD)

            # subfilter weight sums (vector; gpsimd lacks TensorScalarPtr):
            # ws[:,kc,0] = W0+W1, ws[:,kc,1] = W2+W3 — interleaved per kc
            # with the first s-window batch to match DMA arrival order.
            for kc in range(KC):
                nc.vector.scalar_tensor_tensor(
                    out=ws_sb[:, kc, 0], in0=wt_sb[:, kc, 0], scalar=0.0,
                    in1=wt_sb[:, kc, 1], op0=BYPASS, op1=ADD)
                nc.vector.scalar_tensor_tensor(
                    out=ws_sb[:, kc, 1], in0=wt_sb[:, kc, 2], scalar=0.0,
                    in1=wt_sb[:, kc, 3], op0=BYPASS, op1=ADD)
                s_windows(0, [kc])
            s_windows(1)

            f1_prev = {cs: None for cs in range(CS)}  # per-cs previous F1 sbuf

            for g, (kt, cs) in enumerate(groups):
                k0 = kt * 512
                ps = {j: psum_pool.tile([128, 512], F32,
                                        name=f"F{j}_g{g}", tag="ps")
                      for j in range(3)}
                wcol = slice(cs * 128, (cs + 1) * 128)
                for kc in range(KC):
                    # F0 += W0 @ x_e[k] ; F0 += W2 @ x_e[k-1]
                    nc.tensor.matmul(ps[0][:], wt_sb[:, kc, 0, wcol],
                                     xe_sb[:, kc, 1 + k0:1 + k0 + 512],
                                     start=(kc == 0), stop=False)
                    nc.tensor.matmul(ps[0][:], wt_sb[:, kc, 2, wcol],
                                     xe_sb[:, kc, k0:k0 + 512],
                                     start=False, stop=(kc == KC - 1))
                    # F1 += W1 @ x_o[k] ; F1 += W3 @ x_o[k-1]
                    nc.tensor.matmul(ps[1][:], wt_sb[:, kc, 1, wcol],
                                     xo_sb[:, kc, 1 + k0:1 + k0 + 512],
                                     start=(kc == 0), stop=False)
                    nc.tensor.matmul(ps[1][:], wt_sb[:, kc, 3, wcol],
                                     xo_sb[:, kc, k0:k0 + 512],
                                     start=False, stop=(kc == KC - 1))
                    # F2 += (W0+W1) @ s[k] ; F2 += (W2+W3) @ s[k-1]
                    nc.tensor.matmul(ps[2][:], ws_sb[:, kc, 0, wcol],
                                     s_sb[:, kc, 1 + k0:1 + k0 + 512],
                                     start=(kc == 0), stop=False)
                    nc.tensor.matmul(ps[2][:], ws_sb[:, kc, 1, wcol],
                                     s_sb[:, kc, k0:k0 + 512],
                                     start=False, stop=(kc == KC - 1))

                # keep the s pipeline ahead of the matmuls for the next kt
                if cs == 0 and kt + 2 < KT:
                    s_windows(kt + 2)

                # --- recombine ---
                # (DVE reads at most one PSUM operand per op, so F0/F1 are
                # staged to SBUF by the otherwise-idle scalar engine)
                # f1[:, 1+j] = F1[k0+j]; f1[:, 0] = F1[k0-1] (prev tile)
                f1 = fpool.tile([128, 513], F32, name=f"f1_g{g}", tag="f1")
                f0 = fpool.tile([128, 512], F32, name=f"f0_g{g}", tag="f0")
                nc.scalar.copy(f1[:, 1:513], ps[1][:])
                prev = f1_prev[cs]
                if prev is None:
                    nc.scalar.copy(f1[:, 0:1], zcol[:])
                else:
                    nc.scalar.copy(f1[:, 0:1], prev[:, 512:513])
                f1_prev[cs] = f1
                nc.scalar.copy(f0[:], ps[0][:])

                oe = opool.tile([128, 512], F32, name=f"oe_g{g}", tag="ob")
                t1 = opool.tile([128, 512], F32, name=f"t1_g{g}", tag="ob")
                oo = opool.tile([128, 512], F32, name=f"oo_g{g}", tag="ob")
                # y_even = F0 + F1[k-1]
                nc.vector.scalar_tensor_tensor(
                    out=oe[:], in0=f0[:], scalar=0.0, in1=f1[:, 0:512],
                    op0=BYPASS, op1=ADD)
                # y_odd = F2 - F0 - F1
                nc.vector.scalar_tensor_tensor(
                    out=t1[:], in0=f0[:], scalar=-1.0, in1=ps[2][:],
                    op0=MULT, op1=ADD)
                nc.vector.scalar_tensor_tensor(
                    out=oo[:], in0=f1[:, 1:513], scalar=-1.0, in1=t1[:],
                    op0=MULT, op1=ADD)
                nc.scalar.dma_start(out_ct[cs, :, 0, k0:k0 + 512], oe[:])
                nc.scalar.dma_start(out_ct[cs, :, 1, k0:k0 + 512], oo[:])

    nc.compile()
    return nc


def kernel(X: np.ndarray, W: np.ndarray) -> np.ndarray:
    global _last_results
    import ml_dtypes
    bf16 = ml_dtypes.bfloat16
    X = np.ascontiguousarray(X, dtype=np.float32).astype(bf16)
    W = np.ascontiguousarray(W, dtype=np.float32).astype(bf16)

    # phase-split X^T per batch with 1-column causal zero halo:
    # xe[p, kc, 1+k] = X[b, 2k, kc*128+p], xo[p, kc, 1+k] = X[b, 2k+1, ...]
    xes, xos = [], []
    for b in range(BSZ):
        v = X[b].reshape(HT, 2, KC, 128)  # [k, phase, kc, p]
        xe = np.zeros((128, KC, 1 + HT), dtype=bf16)
        xo = np.zeros((128, KC, 1 + HT), dtype=bf16)
        xe[:, :, 1:] = v[:, 0].transpose(2, 1, 0)
        xo[:, :, 1:] = v[:, 1].transpose(2, 1, 0)
        xes.append(xe)
        xos.append(xo)

    # W per core: wt[p, kc, i, c] = W[i*D + cg*CPG + c, kc*128 + p]
    W4 = W.reshape(KW, D, KC, 128)  # [i, d, kc, p]
    wts = []
    for cg in range(CGROUPS):
        wt = W4[:, cg * CPG:(cg + 1) * CPG, :, :].transpose(3, 2, 0, 1)
        wts.append(np.ascontiguousarray(wt))

    global _nc_cache
    if _nc_cache is None:
        _nc_cache = _build_nc()
    nc = _nc_cache
    in_maps = [{"xe": xes[c // CGROUPS], "xo": xos[c // CGROUPS],
                "wt": wts[c % CGROUPS]} for c in range(NCORES)]
    _last_results = run_bass_kernel_spmd(nc, in_maps, core_ids=list(range(NCORES)))

    out = np.empty((BSZ, T, D), dtype=np.float32)
    for c in range(NCORES):
        b, cg = c // CGROUPS, c % CGROUPS
        shard = _last_results.results[c]["out_ct"]  # [CS, 128, 2, HT]
        for cs in range(CS):
            cols = slice(cg * CPG + cs * 128, cg * CPG + (cs + 1) * 128)
            # [128, 2, HT] -> t-interleaved [T, 128]
            out[b, :, cols] = shard[cs].transpose(2, 1, 0).reshape(T, 128)
    return out
